# revision 39
# baseline (speedup 1.0000x reference)
"""Causal multi-head attention on 8 Trainium2 NeuronCores.

Problem: x[2,2048,1024] @ W_Q/K/V[1024,1024] -> 16-head causal attention
(d_head=64) -> @ W_O[1024,1024].

Sharding: tensor-parallel over heads. Core i owns heads 2i, 2i+1 — i.e.
columns [128i:128i+128) of W_Q/W_K/W_V and rows [128i:128i+128) of W_O.
Each core computes its partial output [1024, 4096] (transposed layout, bf16);
the host sums the 8 partials and un-transposes (the "all-reduce").

Device kernel (per core), all matmul operands bf16 (PSUM accumulates fp32):
  1. Projections from xT [1024, 4096] (host pre-transposes + casts bf16):
     Q/K transposed [128, 4096] = W.T @ xT into a fused qkT tile;
     V directly in natural [token, dim] layout (x-chunk as the stationary
     operand), with a ones-column appended per head (65-wide blocks) so the
     PV matmul also produces the softmax denominator for free.
  2. Flash-style causal attention, scores in [k, q] orientation. The causal
     mask is applied INSIDE PSUM by an extra accumulating matmul that adds
     -1e9 above the diagonal (identity stationary x upper-tri mask), so exp
     on ScalarE needs no separate DVE mask multiply. exp is one packed
     [128, 2, live] instruction covering both heads.
  3. Normalization: denominator rows are batch-reciprocated [4, 512] per
     q-tile-pair (instead of 16 serial [1,512] reciprocals), broadcast via
     GPSIMD, and multiplied into bf16 attnT straight out of PSUM.
  4. Output projection interleaved per q-tile pair so the PE never idles
     (keeps the HAM clock-gate warm) and stores batch per q-tile.
"""

import contextlib

import ml_dtypes
import numpy as np

import concourse.bass as bass
import concourse.tile as tile
from concourse import bacc, mybir
from concourse.bass_utils import run_bass_kernel_spmd
from concourse.masks import make_identity


F32 = mybir.dt.float32
BF16 = mybir.dt.bfloat16

N_CORES = 8
P = 128
D = 1024          # d_model
B = 2             # batch
S = 2048          # seq len
T = B * S         # total tokens = 4096
TT = 512          # token tile (free dim of matmuls)
NT = T // TT      # 8 token tiles
KD = D // P       # 8 contraction chunks for projections
JB = S // TT      # 4 q-tiles per batch
CB = S // P       # 16 k-chunks per batch
NCH = T // P      # 32 k-chunks total
H_LOC = 2         # heads per core
DH = 64           # head dim


def _body(tc):
    nc = tc.nc
    xT = nc.dram_tensor("xT", [D, T], BF16, kind="ExternalInput").ap()
    wq = nc.dram_tensor("wq", [D, P], BF16, kind="ExternalInput").ap()
    wk = nc.dram_tensor("wk", [D, P], BF16, kind="ExternalInput").ap()
    wv = nc.dram_tensor("wv", [D, P], BF16, kind="ExternalInput").ap()
    wo = nc.dram_tensor("wo", [P, D], BF16, kind="ExternalInput").ap()
    outT = nc.dram_tensor("outT", [D, T], BF16, kind="ExternalOutput").ap()

    with contextlib.ExitStack() as ctx:
        const = ctx.enter_context(tc.tile_pool(name="const", bufs=1))
        wpool = ctx.enter_context(tc.tile_pool(name="wpool", bufs=1))
        xpool = ctx.enter_context(tc.tile_pool(name="xpool", bufs=4))
        persist = ctx.enter_context(tc.tile_pool(name="persist", bufs=1))
        probs_p = ctx.enter_context(tc.tile_pool(name="probs", bufs=4))
        stage = ctx.enter_context(tc.tile_pool(name="stage", bufs=2))
        bcp = ctx.enter_context(tc.tile_pool(name="bcp", bufs=4))
        obp = ctx.enter_context(tc.tile_pool(name="obp", bufs=3))
        psum = ctx.enter_context(tc.tile_pool(name="psum", bufs=2, space="PSUM"))

        # --- constants -----------------------------------------------------
        identity = const.tile([P, P], BF16)
        make_identity(nc, identity)

        # mask_band[k, q] = 1.0 if q >= k else 0.0 (multiplies probs on the
        # diagonal chunk; cheap bf16 2x-mode DVE op, keeps the PE free)
        mask_band = const.tile([P, P], BF16)
        nc.any.memset(mask_band[:], 1.0)
        nc.gpsimd.affine_select(
            out=mask_band[:],
            in_=mask_band[:],
            compare_op=mybir.AluOpType.is_ge,
            fill=0.0,
            base=0,
            pattern=[[1, P]],
            channel_multiplier=-1,
        )

        # --- weights -------------------------------------------------------
        wq_sb = wpool.tile([P, KD, P], BF16)
        nc.sync.dma_start(wq_sb[:], wq.rearrange("(o p) m -> p o m", p=P))
        wk_sb = wpool.tile([P, KD, P], BF16)
        nc.sync.dma_start(wk_sb[:], wk.rearrange("(o p) m -> p o m", p=P))
        wv_sb = wpool.tile([P, KD, P], BF16)
        nc.sync.dma_start(wv_sb[:], wv.rearrange("(o p) m -> p o m", p=P))
        wo_sb = wpool.tile([P, D], BF16)  # DMA deferred: first use is late

        # --- persistent activations ---------------------------------------
        qkT = persist.tile([P, 2, T], BF16)     # [:,0,:] = QT, [:,1,:] = KT
        vn = persist.tile([P, NCH, 130], BF16)  # [token, chunk, d0|1|d1|1]
        attnT = persist.tile([P, T], BF16)
        # memset (not an activation reading uninitialized SBUF: 0*NaN = NaN
        # would make results depend on leftover SBUF state across runs)
        for col in (DH, 2 * DH + 1):
            nc.any.memset(vn[:, :, col], 1.0)

        xT_r = xT.rearrange("(o p) n -> p o n", p=P)
        outT_r = outT.rearrange("(o p) n -> p o n", p=P)

        # --- phase 1, decomposed into small units that get woven through
        # the attention loop (so ScalarE always has exp work queued and the
        # PE alternates between projections and scores instead of running
        # projections as an ACT-starving block) -----------------------------
        def project_units(t):
            st = {}

            def u_dma():
                st["xt"] = xpool.tile([P, KD, TT], BF16, tag="xt",
                                      name=f"xt_{t}")
                # two halves so the first projection chunks start sooner
                half = KD // 2
                tsl = bass.ts(t, TT)
                nc.sync.dma_start(st["xt"][:, 0:half, :],
                                  xT_r[:, 0:half, tsl])
                nc.sync.dma_start(st["xt"][:, half:KD, :],
                                  xT_r[:, half:KD, tsl])

            def u_proj(g, wsb, dst):
                ps = psum.tile([P, TT], F32, tag="b", name=f"ps{g}_{t}")
                for c in range(KD):
                    nc.tensor.matmul(ps[:], wsb[:, c, :], st["xt"][:, c, :],
                                     start=(c == 0), stop=(c == KD - 1))
                if dst is None:
                    st["vt"] = stage.tile([P, TT], BF16, tag="vt", bufs=4,
                                          name=f"vt_{t}")
                    nc.vector.tensor_copy(st["vt"][:], ps[:])
                else:
                    nc.vector.tensor_copy(dst, ps[:])

            def u_tr(s_):
                ch = t * 4 + s_
                pt = psum.tile([P, P], BF16, tag="b", name=f"pt_{ch}")
                nc.tensor.transpose(pt[:], st["vt"][:, bass.ts(s_, P)],
                                    identity)
                nc.vector.tensor_copy(
                    vn[:, ch, 0:130].rearrange("p (a b) -> p a b", a=2)
                    [:, :, 0:DH],
                    pt[:].rearrange("p (a b) -> p a b", a=2))

            return [
                u_dma,
                lambda: u_proj(0, wq_sb, qkT[:, 0, bass.ts(t, TT)]),
                lambda: u_proj(1, wk_sb, qkT[:, 1, bass.ts(t, TT)]),
                lambda: u_proj(2, wv_sb, None),
                lambda: u_tr(0), lambda: u_tr(1),
                lambda: u_tr(2), lambda: u_tr(3),
            ]

        def project(t):
            for u in project_units(t):
                u()

        # --- phase 2: causal attention + interleaved output projection ----
        # Dual-j: same-index q-tiles of batch 0/1 processed together. Lag-1
        # software pipeline: PV for chunk cb-1 is emitted after the scores
        # for chunk cb so the PE never waits on ScalarE's exp. On diagonal
        # chunks the causal mask is an accumulating matmul adding -1e9.
        def phase3_units(js):
            # output projection for two finished (normalized) q-tiles,
            # decomposed into weavable units
            units = []
            for jx in js:
                jsl = bass.ts(jx, TT)
                st = {}

                def u_wo(fp_, jx=jx, jsl=jsl, st=st):
                    if fp_ == 0:
                        st["ob"] = obp.tile([P, KD, TT], BF16, tag="ob",
                                            name=f"ob_{jx}")
                    wps = psum.tile([P, 2, TT], F32, tag="b",
                                    name=f"wps_{jx}_{fp_}")
                    for g in range(2):
                        f = fp_ * 2 + g
                        nc.tensor.matmul(wps[:, g, :],
                                         wo_sb[:, bass.ts(f, P)],
                                         attnT[:, jsl], start=True, stop=True)
                    nc.vector.tensor_copy(st["ob"][:, fp_ * 2:fp_ * 2 + 2, :],
                                          wps[:])
                    if fp_ == KD // 2 - 1:
                        nc.sync.dma_start(outT_r[:, :, jsl], st["ob"][:])

                for fp_ in range(KD // 2):
                    units.append(lambda fp_=fp_, u=u_wo: u(fp_))
            return units

        def phase3(js):
            for u in phase3_units(js):
                u()

        prev_js = None
        # startup: tiles for the first q-tile pair projected upfront
        project(0)
        project(JB)
        nc.sync.dma_start(wo_sb[:], wo)
        for jj in range(JB):
            # Weave the NEXT pair's projections through this pair's
            # attention loop, a few units per chunk iteration: the PE
            # alternates projection and score matmuls and ScalarE always
            # has exp work, instead of ACT-starving projection blocks at
            # the pair boundaries. (The previous pair's output projection
            # is NOT woven: its PSUM tiles would steal the score tiles'
            # double-buffer slots mid-loop — it runs at the seam instead,
            # where those slots are idle anyway.)
            if jj + 1 < JB:
                # interleave the two tiles' units so each vt copy (DVE) is
                # enqueued several units before the PE transposes that
                # consume it, and both x-tile DMAs prefetch up front
                ua = project_units(jj + 1)
                ub = project_units(jj + 1 + JB)
                punits = [u for pair in zip(ua, ub) for u in pair]
            else:
                punits = []
            pemitted = 0
            ncb = 4 * (jj + 1)
            js = (jj, jj + JB)
            nslots = 2 * ncb
            slot = 0
            pvall = psum.tile([DH + 1, 4, TT], F32, tag="pv", bufs=1,
                              name=f"pv_{jj}")

            def pv_step(jx, cb, pr, jj=jj, ncb=ncb, pvall=pvall):
                b = jx // JB
                c = CB * b + cb
                r = cb - 4 * jj
                lo = P * r if r > 0 else 0
                jloc = 0 if jx == jj else 1
                for h in range(H_LOC):
                    nc.tensor.matmul(pvall[:, jloc * 2 + h, lo:],
                                     vn[:, c, bass.ds((DH + 1) * h, DH + 1)],
                                     pr[:, h, lo:],
                                     start=(cb == 0), stop=(cb == ncb - 1))

            pending = {}
            for cb in range(ncb):
                r = cb - 4 * jj
                lo = P * r if r > 0 else 0
                for jx in js:
                    b = jx // JB
                    c = CB * b + cb
                    csl = bass.ts(c, P)
                    jsl = bass.ts(jx, TT)
                    sps = psum.tile([P, 2, TT], F32, tag="b",
                                    name=f"sps_{jx}_{cb}")
                    for h in range(H_LOC):
                        hp = slice(DH * h, DH * h + DH)
                        nc.tensor.matmul(sps[:, h, lo:], qkT[hp, 1, csl],
                                         qkT[hp, 0, jsl][:, lo:],
                                         start=True, stop=True)
                    pr = probs_p.tile([P, 2, TT], BF16, tag="pr",
                                      name=f"pr_{jx}_{cb}")
                    nc.scalar.activation(pr[:, :, lo:], sps[:, :, lo:],
                                         mybir.ActivationFunctionType.Exp,
                                         scale=0.125)
                    if r >= 0:
                        rsl = bass.ts(r, P)
                        for h in range(H_LOC):
                            nc.vector.tensor_mul(pr[:, h, rsl],
                                                 pr[:, h, rsl], mask_band[:])
                    if jx in pending:
                        pv_step(jx, cb - 1, pending[jx])
                    pending[jx] = pr
                    # weave in the next pair's projection units
                    slot += 1
                    target = slot * len(punits) // nslots
                    while pemitted < target:
                        punits[pemitted]()
                        pemitted += 1
            while pemitted < len(punits):
                punits[pemitted]()
                pemitted += 1
            for jx in js:
                pv_step(jx, ncb - 1, pending[jx])

            # Quick pvall release: unnormalized bf16 casts into attnT plus
            # the denominator row, so the next jj's PV accumulation isn't
            # gated on the (long) reciprocal/broadcast chain.
            dnf = stage.tile([1, 4, TT], F32, tag="dnf", name=f"dnf_{jj}")
            nc.scalar.copy(dnf[:], pvall[DH:DH + 1, :, :])
            for jloc, jx in enumerate(js):
                jsl = bass.ts(jx, TT)
                for h in range(H_LOC):
                    i = jloc * 2 + h
                    hp = slice(DH * h, DH * h + DH)
                    nc.vector.tensor_copy(attnT[hp, jsl], pvall[0:DH, i, :])

            # Deferred normalization (has a whole jj iteration of slack
            # before phase3 of this pair consumes attnT): batch-reciprocal
            # on 4 partitions, one packed partition-broadcast, in-place
            # bf16 multiplies.
            dn = stage.tile([4, TT], F32, tag="dn", name=f"dn_{jj}")
            nc.sync.dma_start(dn[:], dnf[:])
            rdn = stage.tile([4, TT], BF16, tag="rdn", name=f"rdn_{jj}")
            with nc.allow_low_precision(reason="bf16 1/denominator is ample"):
                nc.vector.reciprocal(rdn[:], dn[:])
            rf = stage.tile([1, 4, TT], BF16, tag="rf", name=f"rf_{jj}")
            nc.sync.dma_start(rf[:], rdn[:])
            bc4 = bcp.tile([P, 4, TT], BF16, tag="bc", bufs=2,
                           name=f"bc_{jj}")
            nc.gpsimd.partition_broadcast(bc4[:], rf[:])

            # Previous pair's output projection: ready PE work that fills
            # the seam while this jj's normalization drains on
            # DVE/DMA/GPSIMD (keeps the HAM clock-gate warm).
            if prev_js is not None:
                phase3(prev_js)

            for jloc, jx in enumerate(js):
                jsl = bass.ts(jx, TT)
                for h in range(H_LOC):
                    i = jloc * 2 + h
                    hp = slice(DH * h, DH * h + DH)
                    nc.vector.tensor_mul(attnT[hp, jsl], attnT[hp, jsl],
                                         bc4[hp, i, :])
            prev_js = js
        phase3(prev_js)


_NC_CACHE = None


def _get_nc():
    global _NC_CACHE
    if _NC_CACHE is None:
        nc = bacc.Bacc("TRN2", target_bir_lowering=False, debug=False,
                       num_devices=N_CORES)
        with tile.TileContext(nc) as tc:
            _body(tc)
        nc.compile()
        _NC_CACHE = nc
    return _NC_CACHE


def _in_maps(x, W_Q, W_K, W_V, W_O):
    bf16 = ml_dtypes.bfloat16
    xT = np.ascontiguousarray(
        np.asarray(x, dtype=np.float32).reshape(T, D).T).astype(bf16)
    W_Q = np.asarray(W_Q, dtype=np.float32).astype(bf16)
    W_K = np.asarray(W_K, dtype=np.float32).astype(bf16)
    W_V = np.asarray(W_V, dtype=np.float32).astype(bf16)
    W_O = np.asarray(W_O, dtype=np.float32).astype(bf16)
    maps = []
    for i in range(N_CORES):
        sl = slice(P * i, P * i + P)
        maps.append({
            "xT": xT,
            "wq": np.ascontiguousarray(W_Q[:, sl]),
            "wk": np.ascontiguousarray(W_K[:, sl]),
            "wv": np.ascontiguousarray(W_V[:, sl]),
            "wo": np.ascontiguousarray(W_O[sl, :]),
        })
    return maps


def _gather(results):
    acc = np.zeros([D, T], np.float32)
    for r in results:
        acc += np.asarray(r["outT"]).astype(np.float32)
    return np.ascontiguousarray(acc.T).reshape(B, S, D)


def kernel(x, W_Q, W_K, W_V, W_O):
    nc = _get_nc()
    res = run_bass_kernel_spmd(nc, _in_maps(x, W_Q, W_K, W_V, W_O),
                               core_ids=list(range(N_CORES)))
    return _gather(res.results)


def kernel_profiled(x, W_Q, W_K, W_V, W_O):
    """Like kernel() but with NTFF tracing; returns (output, exec_time_ns)."""
    nc = _get_nc()
    res = run_bass_kernel_spmd(nc, _in_maps(x, W_Q, W_K, W_V, W_O),
                               core_ids=list(range(N_CORES)), trace=True)
    return _gather(res.results), res.exec_time_ns
